# revision 10
# baseline (speedup 1.0000x reference)
"""Trainium2 Bass kernel for nn_CustomerizedLoss (MSE + per-sample weight-conditioned
MLP cross-entropy over a fixed image set).

Sharding: model-batch dim B=64 split across 8 NeuronCores (8 samples each);
the 10000x784 image matrix is replicated (shipped transposed, fp8).

Per core:
  mm1:  h^T[bh=512, n] = W1T[784, 512]^T @ imagesT[784, n]
        K=784 = 6x128 (3 fp8 DoubleRow MMs per bh block) + 16-row remainder
        computed by 4 concurrent row-tiled MMs (tile_position=(32i,0)).
  relu: bias B1 fused into the activation (per-partition bias); split
        3 tiles on ScalarE + 1 tile on VectorE per chunk.
  mm2:  logits[n, 80] = h^T^T @ W2blk[512, 80]   (block-diag W2)
  CE:   one f32 psum read (pb+b2 -> bf16), then max/sub/exp/sum in bf16;
        per-chunk Ln accumulation; one-hot dot via fused tensor_tensor_reduce.
  loss1: (inp1-tar1) on DVE, square+accumulate on ScalarE.
Host combines partial sums into (combined, loss1, loss2).
"""

import numpy as np
import ml_dtypes

BF16 = ml_dtypes.bfloat16
FP8 = ml_dtypes.float8_e4m3

INPUT, HIDDEN, OUT = 784, 64, 10
NTEST, B, WVEC = 10000, 64, 50890
NCORES = 8
BLOC = B // NCORES          # 8 samples per core
BH = BLOC * HIDDEN          # 512
NPAD = 10240                # images padded to 20*512
NCHUNK = 20
CW = 512                    # n-chunk width
KP = 3                      # DoubleRow k-pairs (6 subtiles of 128 rows)
KREM = 16                   # remainder contraction rows (768..783)
L1N = BLOC * WVEC           # 407120
L1COLS = -(-L1N // 128)     # 3181

_CACHE = {}


def _build():
    from contextlib import ExitStack
    import concourse.bass as bass
    from concourse import bacc
    import concourse.mybir as mybir
    import concourse.tile as tile

    f32 = mybir.dt.float32
    bf = mybir.dt.bfloat16
    fp8 = mybir.dt.float8e4
    AX = mybir.AxisListType.X
    OP = mybir.AluOpType
    ACT = mybir.ActivationFunctionType

    nc = bacc.Bacc("TRN2", target_bir_lowering=False, num_devices=NCORES)

    imt_d = nc.declare_dram_parameter("imt", [NCHUNK, 128, 2 * KP, CW], fp8, isOutput=False)
    imr_d = nc.declare_dram_parameter("imr", [KREM, NCHUNK, CW], fp8, isOutput=False)
    w1t_d = nc.declare_dram_parameter("w1t", [128, 2 * KP, BH], fp8, isOutput=False)
    w1r_d = nc.declare_dram_parameter("w1r", [KREM, BH], fp8, isOutput=False)
    b1_d = nc.declare_dram_parameter("b1", [128, 4], f32, isOutput=False)
    w2b_d = nc.declare_dram_parameter("w2b", [128, 4, 80], bf, isOutput=False)
    b2_d = nc.declare_dram_parameter("b2", [128, 320], bf, isOutput=False)
    oh_d = nc.declare_dram_parameter("oh", [NCHUNK, 128, 4 * 8 * 10], bf, isOutput=False)
    mask_d = nc.declare_dram_parameter("mask", [128, 32], f32, isOutput=False)
    x1_d = nc.declare_dram_parameter("x1", [128, L1COLS], bf, isOutput=False)
    t1_d = nc.declare_dram_parameter("t1", [128, L1COLS], bf, isOutput=False)
    out_d = nc.declare_dram_parameter("out", [128, 34], f32, isOutput=True)

    with tile.TileContext(nc) as tc:
        with ExitStack() as ctx:
            persist = ctx.enter_context(tc.tile_pool(name="persist", bufs=1))
            im_pool = ctx.enter_context(tc.tile_pool(name="im", bufs=4))
            oh_pool = ctx.enter_context(tc.tile_pool(name="oh", bufs=4))
            h_pool = ctx.enter_context(tc.tile_pool(name="h", bufs=3))
            s_pool = ctx.enter_context(tc.tile_pool(name="s", bufs=3))
            pa_pool = ctx.enter_context(tc.tile_pool(name="pa", bufs=1, space="PSUM"))
            pb_pool = ctx.enter_context(tc.tile_pool(name="pb", bufs=4, space="PSUM"))

            # preload the ACT table set containing exp+ln+relu+square (id 6)
            nc.scalar.add_instruction(mybir.InstLoadActFuncSet(
                name=nc.get_next_instruction_name(), ins=[], outs=[],
                act_func_set_id=6))

            # critical-path DMAs first: chunk-0 image pairs + weight pairs
            im0 = [persist.tile([128, 2, CW], fp8, name=f"im0_{k}") for k in range(KP)]
            w1tP = [persist.tile([128, 2, BH], fp8, name=f"w1tP{k}") for k in range(KP)]
            for k in range(KP):
                nc.sync.dma_start(out=im0[k], in_=imt_d[0, :, 2 * k:2 * k + 2, :])
                nc.sync.dma_start(out=w1tP[k], in_=w1t_d[:, 2 * k:2 * k + 2, :])
            b1 = persist.tile([128, 4], f32)
            nc.sync.dma_start(out=b1, in_=b1_d[:, :])
            w1r = persist.tile([KREM, BH], fp8)
            nc.sync.dma_start(out=w1r, in_=w1r_d[:, :])
            imr = persist.tile([KREM, NCHUNK, CW], fp8)
            nc.sync.dma_start(out=imr, in_=imr_d[:, :, :])
            w2b = persist.tile([128, 4, 80], bf)
            nc.sync.dma_start(out=w2b, in_=w2b_d[:, :, :])
            b2 = persist.tile([128, 320], bf)
            nc.sync.dma_start(out=b2, in_=b2_d[:, :])
            mask = persist.tile([128, 32], f32)
            nc.sync.dma_start(out=mask, in_=mask_d[:, :])

            lacc = persist.tile([128, 32], f32)
            nc.gpsimd.memset(lacc, 0.0)
            macc = persist.tile([128, 32], f32)
            nc.gpsimd.memset(macc, 0.0)
            dotv_all = persist.tile([128, NCHUNK], f32)
            outt = persist.tile([128, 34], f32)

            x1 = persist.tile([128, L1COLS], bf)
            t1 = persist.tile([128, L1COLS], bf)

            for c in range(NCHUNK):
                if c == 0:
                    ims = im0
                else:
                    im = im_pool.tile([128, 2 * KP, CW], fp8)
                    nc.sync.dma_start(out=im, in_=imt_d[c, :, :, :])
                    ims = [im[:, 2 * k:2 * k + 2, :] for k in range(KP)]
                oht = oh_pool.tile([128, 32, 10], bf)
                nc.sync.dma_start(
                    out=oht.rearrange("p g o -> p (g o)"), in_=oh_d[c, :, :]
                )
                if c == 1:
                    nc.sync.dma_start(out=x1, in_=x1_d[:, :])
                    nc.sync.dma_start(out=t1, in_=t1_d[:, :])

                # mm1: 3 fp8 DoubleRow MMs per bh block (K rows 0..767)
                pas = []
                for bh in range(4):
                    pa = pa_pool.tile([128, CW], f32, name=f"pa{bh}_{c}", tag=f"pa{bh}")
                    for k in range(KP):
                        nc.tensor.matmul(
                            pa[:, :],
                            w1tP[k][:, :, bh * 128:(bh + 1) * 128],
                            ims[k],
                            start=(k == 0), stop=False,
                            perf_mode=mybir.MatmulPerfMode.DoubleRow,
                        )
                    pas.append(pa)
                # K remainder rows 768..783 per bh block
                for bh in range(4):
                    nc.tensor.matmul(
                        pas[bh][:, :],
                        w1r[:, bh * 128:(bh + 1) * 128],
                        imr[:, c, :],
                        start=False, stop=True,
                    )

                # relu with fused per-partition bias B1; 3 on ACT + 1 on DVE
                hts = [h_pool.tile([128, CW], bf, name=f"ht{j}_{c}", tag=f"ht{j}") for j in range(4)]
                nc.vector.tensor_scalar(
                    out=hts[0], in0=pas[0][:, :],
                    scalar1=b1[:, 0:1], scalar2=0.0,
                    op0=OP.add, op1=OP.max,
                )
                for j in range(1, 4):
                    nc.scalar.activation(
                        out=hts[j], in_=pas[j][:, :], func=ACT.Relu,
                        bias=b1[:, j:j + 1],
                    )

                # mm2: block-diag W2, 16 MMs of N=80
                pb = pb_pool.tile([128, 32, 10], f32)
                for ns in range(4):
                    outap = pb[:, ns * 8:(ns + 1) * 8, :].rearrange("p g o -> p (g o)")
                    for j in range(4):
                        nc.tensor.matmul(
                            outap,
                            hts[j][:, ns * 128:(ns + 1) * 128],
                            w2b[:, j, :],
                            start=(j == 0), stop=(j == 3),
                        )

                # CE: single f32 psum read, then bf16 chain
                pbb = s_pool.tile([128, 32, 10], bf)
                nc.vector.tensor_tensor(
                    pbb.rearrange("p g o -> p (g o)"),
                    pb.rearrange("p g o -> p (g o)"),
                    b2, OP.add,
                )
                mx = s_pool.tile([128, 32], bf)
                nc.vector.tensor_reduce(out=mx, in_=pbb, axis=AX, op=OP.max)
                S = s_pool.tile([128, 32, 10], bf)
                nc.vector.tensor_tensor(
                    S, pbb, mx[:, :, None].broadcast_to([128, 32, 10]), OP.subtract
                )
                E = s_pool.tile([128, 32, 10], bf)
                nc.scalar.activation(
                    out=E.rearrange("p g o -> p (g o)"),
                    in_=S.rearrange("p g o -> p (g o)"), func=ACT.Exp,
                )
                ssum = s_pool.tile([128, 32], f32)
                nc.vector.tensor_reduce(out=ssum, in_=E, axis=AX, op=OP.add)
                lnc = s_pool.tile([128, 32], f32)
                nc.scalar.activation(out=lnc, in_=ssum, func=ACT.Ln)
                if c == NCHUNK - 1:
                    # mask out padded images in the last chunk
                    lnm = s_pool.tile([128, 32], f32)
                    nc.vector.tensor_tensor(lnm, lnc, mask, OP.mult)
                    mxm = s_pool.tile([128, 32], f32)
                    nc.vector.tensor_tensor(mxm, mx, mask, OP.mult)
                    nc.vector.tensor_add(lacc, lacc, lnm)
                    nc.vector.tensor_add(macc, macc, mxm)
                else:
                    nc.vector.tensor_add(lacc, lacc, lnc)
                    nc.vector.tensor_add(macc, macc, mx)
                # one-hot target dot
                junk = s_pool.tile([128, 32, 10], bf)
                nc.vector.tensor_tensor(
                    junk.rearrange("p g o -> p (g o)"),
                    pbb.rearrange("p g o -> p (g o)"),
                    oht.rearrange("p g o -> p (g o)"),
                    OP.mult,
                )
                nc.vector.tensor_reduce(
                    out=dotv_all[:, c:c + 1], in_=junk,
                    axis=mybir.AxisListType.XY, op=OP.add,
                )

                if c == 2:
                    # loss1: d = inp1-tar1 (DVE), square (ACT), reduce (DVE)
                    nc.vector.tensor_sub(x1, x1, t1)
                    nc.scalar.activation(out=t1, in_=x1, func=ACT.Square)
                    nc.vector.tensor_reduce(
                        out=outt[:, 33:34], in_=t1, axis=AX, op=OP.add,
                    )

            # finale
            nc.vector.tensor_reduce(out=outt[:, 32:33], in_=dotv_all, axis=AX, op=OP.add)
            nc.vector.tensor_tensor(outt[:, 0:32], lacc, macc, OP.add)
            nc.sync.dma_start(out=out_d[:, :], in_=outt)

    nc.compile()
    return nc


def _prep_shared(images):
    """imagesT in fp8, split into 6x128-row k-subtiles per 512-col chunk
    (imt [NCHUNK, 128, 6, CW]) plus the 16-row remainder replicated into
    the four 32-row strips (imr [128, NCHUNK, CW])."""
    imT = np.zeros((INPUT, NPAD), dtype=np.float32)
    imT[:, :NTEST] = images.T
    main = imT[:768].reshape(6, 128, NCHUNK, CW).transpose(2, 1, 0, 3)
    imt = np.ascontiguousarray(main.astype(FP8))  # [NCHUNK, 128, 6, CW]
    remr = imT[768:INPUT].reshape(KREM, NCHUNK, CW)
    return imt, np.ascontiguousarray(remr.astype(FP8))


def _prep_core(inp1, tar1, inp2, tar2):
    """Per-core input dict from this core's 8-sample slices."""
    o1 = INPUT * HIDDEN
    o2 = o1 + HIDDEN
    o3 = o2 + HIDDEN * OUT
    W1 = inp2[:, :o1].reshape(BH, INPUT)
    B1 = inp2[:, o1:o2].reshape(BH)
    W2 = inp2[:, o2:o3].reshape(BLOC, OUT, HIDDEN)
    B2 = inp2[:, o3:].reshape(1, BLOC * OUT)

    # main K rows 0..767: [128, 6, BH]
    w1t = W1[:, :768].reshape(BH, 6, 128).transpose(2, 1, 0)
    # remainder rows 768..783: [16, BH]
    w1r = W1[:, 768:INPUT].T.copy()
    b1 = B1.reshape(4, 128).T  # [128, 4]

    w2blk = np.zeros((BH, BLOC * OUT), dtype=np.float32)
    for b in range(BLOC):
        w2blk[b * HIDDEN:(b + 1) * HIDDEN, b * OUT:(b + 1) * OUT] = W2[b].T
    w2b = w2blk.reshape(4, 128, 80).transpose(1, 0, 2)

    # one-hot labels in device layout [NCHUNK, 128, 4*8*10]
    oh = np.zeros((BLOC, NPAD, OUT), dtype=np.float32)
    oh[np.arange(BLOC)[:, None], np.arange(NTEST)[None, :], tar2.astype(np.int64)] = 1.0
    # [b, chunk, ns, p, o] -> [chunk, p, ns, b, o]
    ohd = oh.reshape(BLOC, NCHUNK, 4, 128, OUT).transpose(1, 3, 2, 0, 4)
    ohd = ohd.reshape(NCHUNK, 128, 4 * BLOC * OUT)

    mask = np.zeros((128, 32), dtype=np.float32)
    n0 = (NCHUNK - 1) * CW
    for ns in range(4):
        valid = np.clip(NTEST - (n0 + ns * 128), 0, 128)
        mask[:valid, ns * 8:(ns + 1) * 8] = 1.0

    x1 = np.zeros((128 * L1COLS,), dtype=np.float32)
    x1[:L1N] = inp1.ravel()
    t1 = np.zeros((128 * L1COLS,), dtype=np.float32)
    t1[:L1N] = tar1.ravel()

    return {
        "w1t": np.ascontiguousarray(w1t.astype(FP8)),
        "w1r": np.ascontiguousarray(w1r.astype(FP8)),
        "b1": np.ascontiguousarray(b1.astype(np.float32)),
        "w2b": np.ascontiguousarray(w2b.astype(BF16)),
        "b2": np.ascontiguousarray(np.tile(B2.reshape(-1), (128, 4)).astype(BF16)),
        "oh": np.ascontiguousarray(ohd.astype(BF16)),
        "mask": mask,
        "x1": x1.reshape(128, L1COLS).astype(BF16),
        "t1": t1.reshape(128, L1COLS).astype(BF16),
    }


def kernel(inp1, tar1, inp2, tar2, images, _want_results=False):
    from concourse.bass_utils import run_bass_kernel_spmd

    inp1 = np.asarray(inp1, dtype=np.float32)
    tar1 = np.asarray(tar1, dtype=np.float32)
    inp2 = np.asarray(inp2, dtype=np.float32)
    tar2 = np.asarray(tar2)
    images = np.asarray(images, dtype=np.float32)

    if "nc" not in _CACHE:
        _CACHE["nc"] = _build()
    nc = _CACHE["nc"]

    imt, imr = _prep_shared(images)
    in_maps = []
    for core in range(NCORES):
        s = slice(core * BLOC, (core + 1) * BLOC)
        m = _prep_core(inp1[s], tar1[s], inp2[s], tar2[s])
        m["imt"] = imt
        m["imr"] = imr
        in_maps.append(m)

    res = run_bass_kernel_spmd(nc, in_maps, core_ids=list(range(NCORES)))

    ce_sum = 0.0
    sq_sum = 0.0
    for core in range(NCORES):
        o = res.results[core]["out"].astype(np.float64)
        ce_sum += np.sum(o[:, 0:32]) - np.sum(o[:, 32])
        sq_sum += np.sum(o[:, 33])

    loss1 = 20.0 * sq_sum / (B * WVEC)
    loss2 = ce_sum / (B * NTEST)
    combined = loss1 + loss2
    out = (
        np.float32(combined),
        np.float32(loss1),
        np.float32(loss2),
    )
    if _want_results:
        return out, res
    return out


# revision 19
# speedup vs baseline: 1.1061x; 1.1061x over previous
"""Trainium2 Bass kernel for nn_CustomerizedLoss (MSE + per-sample weight-conditioned
MLP cross-entropy over a fixed image set).

Sharding: model-batch dim B=64 split across 8 NeuronCores (8 samples each);
the 10000x784 image matrix is replicated (shipped transposed, fp8).

Per core:
  mm1:  h^T[bh=512, n] = W1T[784, 512]^T @ imagesT[784, n]
        K=784 = 6x128 (3 fp8 DoubleRow MMs per bh block) + 16-row remainder
        computed by 4 concurrent row-tiled MMs (tile_position=(32i,0)).
  relu: bias B1 fused into the activation (per-partition bias); split
        3 tiles on ScalarE + 1 tile on VectorE per chunk.
  mm2:  logits[n, 80] = h^T^T @ W2blk[512, 80]   (block-diag W2)
  CE:   one f32 psum read (pb+b2 -> bf16), then max/sub/exp/sum in bf16;
        per-chunk Ln accumulation; one-hot dot via fused tensor_tensor_reduce.
  loss1: (inp1-tar1) on DVE, square+accumulate on ScalarE.
Host combines partial sums into (combined, loss1, loss2).
"""

import numpy as np
import ml_dtypes

BF16 = ml_dtypes.bfloat16
FP8 = ml_dtypes.float8_e4m3

INPUT, HIDDEN, OUT = 784, 64, 10
NTEST, B, WVEC = 10000, 64, 50890
NCORES = 8
BLOC = B // NCORES          # 8 samples per core
BH = BLOC * HIDDEN          # 512
NPAD = 10240                # images padded to 20*512
NCHUNK = 20
CW = 512                    # n-chunk width
KP = 3                      # DoubleRow k-pairs (6 subtiles of 128 rows)
KREM = 16                   # remainder contraction rows (768..783)
L1N = BLOC * WVEC           # 407120
L1COLS = -(-L1N // 128)     # 3181

_CACHE = {}


def _build():
    from contextlib import ExitStack
    import concourse.bass as bass
    from concourse import bacc
    import concourse.mybir as mybir
    import concourse.tile as tile

    f32 = mybir.dt.float32
    bf = mybir.dt.bfloat16
    fp8 = mybir.dt.float8e4
    AX = mybir.AxisListType.X
    OP = mybir.AluOpType
    ACT = mybir.ActivationFunctionType

    nc = bacc.Bacc("TRN2", target_bir_lowering=False, num_devices=NCORES)

    imt_d = nc.declare_dram_parameter("imt", [NCHUNK, 128, 2 * KP, CW], fp8, isOutput=False)
    imr_d = nc.declare_dram_parameter("imr", [128, NCHUNK, CW], fp8, isOutput=False)
    w1t_d = nc.declare_dram_parameter("w1t", [128, 2 * KP, BH], fp8, isOutput=False)
    w1r_d = nc.declare_dram_parameter("w1r", [128, 128], fp8, isOutput=False)
    b1_d = nc.declare_dram_parameter("b1", [128, 4], f32, isOutput=False)
    w2b_d = nc.declare_dram_parameter("w2b", [128, 4, 80], bf, isOutput=False)
    b2_d = nc.declare_dram_parameter("b2", [128, 320], bf, isOutput=False)
    oh_d = nc.declare_dram_parameter("oh", [NCHUNK, 128, 4 * 8 * 10], bf, isOutput=False)
    mask_d = nc.declare_dram_parameter("mask", [128, 32], f32, isOutput=False)
    x1_d = nc.declare_dram_parameter("x1", [128, L1COLS], bf, isOutput=False)
    t1_d = nc.declare_dram_parameter("t1", [128, L1COLS], bf, isOutput=False)
    out_d = nc.declare_dram_parameter("out", [128, 34], f32, isOutput=True)

    with tile.TileContext(nc) as tc:
        with ExitStack() as ctx:
            persist = ctx.enter_context(tc.tile_pool(name="persist", bufs=1))
            im_pool = ctx.enter_context(tc.tile_pool(name="im", bufs=4))
            oh_pool = ctx.enter_context(tc.tile_pool(name="oh", bufs=4))
            h_pool = ctx.enter_context(tc.tile_pool(name="h", bufs=3))
            s_pool = ctx.enter_context(tc.tile_pool(name="s", bufs=3))
            pa_pool = ctx.enter_context(tc.tile_pool(name="pa", bufs=1, space="PSUM"))
            pa3_pool = ctx.enter_context(tc.tile_pool(name="pa3", bufs=2, space="PSUM"))
            pb_pool = ctx.enter_context(tc.tile_pool(name="pb", bufs=2, space="PSUM"))

            # preload the ACT table set containing exp+ln+relu+square (id 6)
            nc.scalar.add_instruction(mybir.InstLoadActFuncSet(
                name=nc.get_next_instruction_name(), ins=[], outs=[],
                act_func_set_id=6))

            # critical-path DMAs first: chunk-0 image pairs + weight pairs
            im0 = [persist.tile([128, 2, CW], fp8, name=f"im0_{k}") for k in range(KP)]
            w1tP = [persist.tile([128, 2, BH], fp8, name=f"w1tP{k}") for k in range(KP)]
            for k in range(KP):
                nc.sync.dma_start(out=im0[k], in_=imt_d[0, :, 2 * k:2 * k + 2, :])
                nc.sync.dma_start(out=w1tP[k], in_=w1t_d[:, 2 * k:2 * k + 2, :])
            b1 = persist.tile([128, 4], f32)
            nc.sync.dma_start(out=b1, in_=b1_d[:, :])
            w1r = persist.tile([128, 128], fp8)
            nc.sync.dma_start(out=w1r, in_=w1r_d[:, :])
            imr = persist.tile([128, NCHUNK, CW], fp8)
            nc.sync.dma_start(out=imr, in_=imr_d[:, :, :])
            w2b = persist.tile([128, 4, 80], bf)
            nc.sync.dma_start(out=w2b, in_=w2b_d[:, :, :])
            b2 = persist.tile([128, 320], bf)
            nc.sync.dma_start(out=b2, in_=b2_d[:, :])
            mask = persist.tile([128, 32], f32)
            nc.sync.dma_start(out=mask, in_=mask_d[:, :])

            lacc = persist.tile([128, 32], f32)
            nc.gpsimd.memset(lacc, 0.0)
            macc = persist.tile([128, 32], f32)
            nc.gpsimd.memset(macc, 0.0)
            dotv_all = persist.tile([128, NCHUNK], f32)
            outt = persist.tile([128, 34], f32)

            x1 = persist.tile([128, L1COLS], bf)
            t1 = persist.tile([128, L1COLS], bf)

            for c in range(NCHUNK):
                if c == 0:
                    ims = im0
                else:
                    im = im_pool.tile([128, 2 * KP, CW], fp8)
                    nc.sync.dma_start(out=im, in_=imt_d[c, :, :, :])
                    ims = [im[:, 2 * k:2 * k + 2, :] for k in range(KP)]
                oht = oh_pool.tile([128, 32, 10], bf)
                nc.sync.dma_start(
                    out=oht.rearrange("p g o -> p (g o)"), in_=oh_d[c, :, :]
                )
                if c == 1:
                    nc.sync.dma_start(out=x1, in_=x1_d[:, :])
                    nc.sync.dma_start(out=t1, in_=t1_d[:, :])

                # mm1: 3 fp8 DoubleRow MMs per bh block (K rows 0..767)
                pas = []
                for bh in range(4):
                    pool = pa3_pool if bh == 3 else pa_pool
                    pa = pool.tile([128, CW], f32, name=f"pa{bh}_{c}", tag=f"pa{bh}")
                    for k in range(KP):
                        nc.tensor.matmul(
                            pa[:, :],
                            w1tP[k][:, :, bh * 128:(bh + 1) * 128],
                            ims[k],
                            start=(k == 0), stop=False,
                            perf_mode=mybir.MatmulPerfMode.DoubleRow,
                        )
                    pas.append(pa)
                # K remainder rows 768..783: 4 concurrent row-tiled MMs
                for bh in range(4):
                    nc.tensor.matmul(
                        pas[bh][:, :],
                        w1r[32 * bh:32 * bh + KREM, :],
                        imr[32 * bh:32 * bh + KREM, c, :],
                        start=False, stop=True,
                        tile_position=(32 * bh, 0),
                    )

                # relu with fused per-partition bias B1; 3 on ACT + 1 on DVE
                # (bh=3 on DVE: its psum bank is double-buffered so a DVE
                # backlog cannot stall the next chunk's matmuls)
                hts = [h_pool.tile([128, CW], bf, name=f"ht{j}_{c}", tag=f"ht{j}") for j in range(4)]
                for j in range(3):
                    nc.scalar.activation(
                        out=hts[j], in_=pas[j][:, :], func=ACT.Relu,
                        bias=b1[:, j:j + 1],
                    )
                nc.vector.tensor_scalar(
                    out=hts[3], in0=pas[3][:, :],
                    scalar1=b1[:, 3:4], scalar2=0.0,
                    op0=OP.add, op1=OP.max,
                )

                # mm2: block-diag W2, 16 MMs of N=80
                pb = pb_pool.tile([128, 32, 10], f32)
                for ns in range(4):
                    outap = pb[:, ns * 8:(ns + 1) * 8, :].rearrange("p g o -> p (g o)")
                    for j in range(4):
                        nc.tensor.matmul(
                            outap,
                            hts[j][:, ns * 128:(ns + 1) * 128],
                            w2b[:, j, :],
                            start=(j == 0), stop=(j == 3),
                        )

                # CE: single f32 psum read, then bf16 chain
                pbb = s_pool.tile([128, 32, 10], bf)
                nc.vector.tensor_tensor(
                    pbb.rearrange("p g o -> p (g o)"),
                    pb.rearrange("p g o -> p (g o)"),
                    b2, OP.add,
                )
                mx = s_pool.tile([128, 32], bf)
                nc.vector.tensor_reduce(out=mx, in_=pbb, axis=AX, op=OP.max)
                S = s_pool.tile([128, 32, 10], bf)
                nc.vector.tensor_tensor(
                    S, pbb, mx[:, :, None].broadcast_to([128, 32, 10]), OP.subtract
                )
                E = s_pool.tile([128, 32, 10], bf)
                nc.scalar.activation(
                    out=E.rearrange("p g o -> p (g o)"),
                    in_=S.rearrange("p g o -> p (g o)"), func=ACT.Exp,
                )
                ssum = s_pool.tile([128, 32], f32)
                nc.vector.tensor_reduce(out=ssum, in_=E, axis=AX, op=OP.add)
                lnc = s_pool.tile([128, 32], f32)
                nc.scalar.activation(out=lnc, in_=ssum, func=ACT.Ln)
                if c == NCHUNK - 1:
                    # mask out padded images in the last chunk
                    lnm = s_pool.tile([128, 32], f32)
                    nc.gpsimd.tensor_tensor(lnm, lnc, mask, OP.mult)
                    mxm = s_pool.tile([128, 32], f32)
                    nc.gpsimd.tensor_tensor(mxm, mx, mask, OP.mult)
                    nc.gpsimd.tensor_add(lacc, lacc, lnm)
                    nc.gpsimd.tensor_add(macc, macc, mxm)
                else:
                    nc.gpsimd.tensor_add(lacc, lacc, lnc)
                    nc.gpsimd.tensor_add(macc, macc, mx)
                # one-hot target dot (mult on GpSimd, reduce on DVE)
                junk = s_pool.tile([128, 32, 10], bf)
                nc.gpsimd.tensor_tensor(
                    junk.rearrange("p g o -> p (g o)"),
                    pbb.rearrange("p g o -> p (g o)"),
                    oht.rearrange("p g o -> p (g o)"),
                    OP.mult,
                )
                nc.vector.tensor_reduce(
                    out=dotv_all[:, c:c + 1], in_=junk,
                    axis=mybir.AxisListType.XY, op=OP.add,
                )

                if c == 2:
                    # loss1: d = inp1-tar1 (DVE), square (ACT), reduce (DVE)
                    nc.vector.tensor_sub(x1, x1, t1)
                    nc.scalar.activation(out=t1, in_=x1, func=ACT.Square)
                    nc.vector.tensor_reduce(
                        out=outt[:, 33:34], in_=t1, axis=AX, op=OP.add,
                    )

            # finale
            nc.vector.tensor_reduce(out=outt[:, 32:33], in_=dotv_all, axis=AX, op=OP.add)
            nc.vector.tensor_tensor(outt[:, 0:32], lacc, macc, OP.add)
            nc.sync.dma_start(out=out_d[:, :], in_=outt)

    nc.compile()
    return nc


def _prep_shared(images):
    """imagesT in fp8, split into 6x128-row k-subtiles per 512-col chunk
    (imt [NCHUNK, 128, 6, CW]) plus the 16-row remainder replicated into
    the four 32-row strips (imr [128, NCHUNK, CW])."""
    imT = np.zeros((INPUT, NPAD), dtype=np.float32)
    imT[:, :NTEST] = images.T
    main = imT[:768].reshape(6, 128, NCHUNK, CW).transpose(2, 1, 0, 3)
    imt = np.ascontiguousarray(main.astype(FP8))  # [NCHUNK, 128, 6, CW]
    remr = imT[768:INPUT].reshape(KREM, NCHUNK, CW)
    imr = np.zeros((128, NCHUNK, CW), dtype=np.float32)
    for i in range(4):
        imr[32 * i:32 * i + KREM] = remr
    return imt, np.ascontiguousarray(imr.astype(FP8))


def _prep_core(inp1, tar1, inp2, tar2):
    """Per-core input dict from this core's 8-sample slices."""
    o1 = INPUT * HIDDEN
    o2 = o1 + HIDDEN
    o3 = o2 + HIDDEN * OUT
    W1 = inp2[:, :o1].reshape(BH, INPUT)
    B1 = inp2[:, o1:o2].reshape(BH)
    W2 = inp2[:, o2:o3].reshape(BLOC, OUT, HIDDEN)
    B2 = inp2[:, o3:].reshape(1, BLOC * OUT)

    # main K rows 0..767: [128, 6, BH]
    w1t = W1[:, :768].reshape(BH, 6, 128).transpose(2, 1, 0)
    # remainder rows 768..783 per bh block i at partitions 32i..32i+15
    w1r = np.zeros((128, 128), dtype=np.float32)
    for i in range(4):
        w1r[32 * i:32 * i + KREM, :] = W1[i * 128:(i + 1) * 128, 768:INPUT].T
    b1 = B1.reshape(4, 128).T  # [128, 4]

    w2blk = np.zeros((BH, BLOC * OUT), dtype=np.float32)
    for b in range(BLOC):
        w2blk[b * HIDDEN:(b + 1) * HIDDEN, b * OUT:(b + 1) * OUT] = W2[b].T
    w2b = w2blk.reshape(4, 128, 80).transpose(1, 0, 2)

    # one-hot labels in device layout [NCHUNK, 128, 4*8*10]
    oh = np.zeros((BLOC, NPAD, OUT), dtype=np.float32)
    oh[np.arange(BLOC)[:, None], np.arange(NTEST)[None, :], tar2.astype(np.int64)] = 1.0
    # [b, chunk, ns, p, o] -> [chunk, p, ns, b, o]
    ohd = oh.reshape(BLOC, NCHUNK, 4, 128, OUT).transpose(1, 3, 2, 0, 4)
    ohd = ohd.reshape(NCHUNK, 128, 4 * BLOC * OUT)

    mask = np.zeros((128, 32), dtype=np.float32)
    n0 = (NCHUNK - 1) * CW
    for ns in range(4):
        valid = np.clip(NTEST - (n0 + ns * 128), 0, 128)
        mask[:valid, ns * 8:(ns + 1) * 8] = 1.0

    x1 = np.zeros((128 * L1COLS,), dtype=np.float32)
    x1[:L1N] = inp1.ravel()
    t1 = np.zeros((128 * L1COLS,), dtype=np.float32)
    t1[:L1N] = tar1.ravel()

    return {
        "w1t": np.ascontiguousarray(w1t.astype(FP8)),
        "w1r": np.ascontiguousarray(w1r.astype(FP8)),
        "b1": np.ascontiguousarray(b1.astype(np.float32)),
        "w2b": np.ascontiguousarray(w2b.astype(BF16)),
        "b2": np.ascontiguousarray(np.tile(B2.reshape(-1), (128, 4)).astype(BF16)),
        "oh": np.ascontiguousarray(ohd.astype(BF16)),
        "mask": mask,
        "x1": x1.reshape(128, L1COLS).astype(BF16),
        "t1": t1.reshape(128, L1COLS).astype(BF16),
    }


def kernel(inp1, tar1, inp2, tar2, images, _want_results=False):
    from concourse.bass_utils import run_bass_kernel_spmd

    inp1 = np.asarray(inp1, dtype=np.float32)
    tar1 = np.asarray(tar1, dtype=np.float32)
    inp2 = np.asarray(inp2, dtype=np.float32)
    tar2 = np.asarray(tar2)
    images = np.asarray(images, dtype=np.float32)

    if "nc" not in _CACHE:
        _CACHE["nc"] = _build()
    nc = _CACHE["nc"]

    imt, imr = _prep_shared(images)
    in_maps = []
    for core in range(NCORES):
        s = slice(core * BLOC, (core + 1) * BLOC)
        m = _prep_core(inp1[s], tar1[s], inp2[s], tar2[s])
        m["imt"] = imt
        m["imr"] = imr
        in_maps.append(m)

    res = run_bass_kernel_spmd(nc, in_maps, core_ids=list(range(NCORES)))

    ce_sum = 0.0
    sq_sum = 0.0
    for core in range(NCORES):
        o = res.results[core]["out"].astype(np.float64)
        ce_sum += np.sum(o[:, 0:32]) - np.sum(o[:, 32])
        sq_sum += np.sum(o[:, 33])

    loss1 = 20.0 * sq_sum / (B * WVEC)
    loss2 = ce_sum / (B * NTEST)
    combined = loss1 + loss2
    out = (
        np.float32(combined),
        np.float32(loss1),
        np.float32(loss2),
    )
    if _want_results:
        return out, res
    return out
